# revision 35
# baseline (speedup 1.0000x reference)
"""MoE grouped-GEMM (Experts) kernel for 8 Trainium2 NeuronCores.

Problem: x [8192, 1024] f32, tokens grouped contiguously by expert (1024
tokens per expert), weight [8, 4096, 1024], bias [8, 4096].  Output
y[t] = x[t] @ W[e(t)].T + b[e(t)], shape [8192, 4096].

Sharding: expert parallelism — expert e lives on core e.  Token dispatch is
a host-side slice (tokens are pre-sorted and balanced), so no collectives
are needed.  Each core runs one [1024x1024] @ [1024x4096] GEMM.

Device layout: the host feeds each core xT = x_e.T [DIN, TE] and
wT = W_e.T [DIN, DOUT] (both contiguous) so every DMA is wide-row
contiguous and the contraction dim K=DIN lands on SBUF partitions with no
on-device transposes.  Per core:
  - xT is SBUF-resident, wT streams through SBUF in 512-col chunks.
  - For each n-chunk j (512 cols) and each m-tile (128 tokens):
    8 accumulating matmuls over the K tiles into one PSUM bank,
    then a DVE add of the (partition-broadcast) bias evicts PSUM->SBUF,
    and the result DMAs out.  PSUM banks rotate (bufs=8) so the PE never
    stalls on eviction.

Matmul dtype: plain fp32 matmul on TRN2 is decomposed by the compiler
into 2 half-rate passes (4 cycles/row) — 4x slower than bf16.  float32r
(TF32-like rounding, fp32 storage) streams at 1 cycle/row for moving
free dim >= 256.  The matmul input dtype is configurable below.
"""

import sys

for _p in ("/opt/trn_rl_repo",):
    if _p not in sys.path:
        sys.path.insert(0, _p)

import numpy as np

import concourse.bass as bass  # noqa: F401  (registers lowerings)
import concourse.mybir as mybir
import concourse.tile as tile
from concourse import bacc
from concourse.bass_utils import run_bass_kernel_spmd

E = 8
T = 8192
DIN = 1024
DOUT = 4096
TE = T // E          # 1024 tokens per expert
P = 128
KT = DIN // P        # 8 k-tiles
MT = TE // P         # 8 m-tiles
NFREE = 512          # matmul moving free dim (one PSUM bank of fp32)
NCH = DOUT // NFREE  # 8 n-chunks
F32 = mybir.dt.float32

# Matmul input dtype for the device kernel.
MM_DTYPE = mybir.dt.float32r

_NC_CACHE = {}


def _np_dtype(mm_dtype):
    return mybir.dt.np(mm_dtype)


def _build_nc(mm_dtype):
    nc = bacc.Bacc(None, target_bir_lowering=False)

    xt = nc.dram_tensor("xt", [DIN, TE], mm_dtype, kind="ExternalInput")
    wt = nc.dram_tensor("wt", [DIN, DOUT], mm_dtype, kind="ExternalInput")
    bi = nc.dram_tensor("bi", [1, DOUT], F32, kind="ExternalInput")
    y = nc.dram_tensor("y", [TE, DOUT], F32, kind="ExternalOutput")

    with tile.TileContext(nc) as tc:
        with (
            tc.tile_pool(name="xp", bufs=1) as xp,
            tc.tile_pool(name="wp", bufs=3 * KT) as wp,
            tc.tile_pool(name="bp", bufs=1) as bp,
            tc.tile_pool(name="op", bufs=4) as op,
            tc.tile_pool(name="pp", bufs=8, space="PSUM") as pp,
        ):
            # Bias: one row DMA + partition broadcast to all 128 partitions.
            # Loaded via the GPSIMD SWDGE so it stays off the Sync engine's
            # serial dma_start dispatch path (~640 ns each); the broadcast
            # result is only needed at the first PSUM eviction (~16 us in).
            brow = bp.tile([1, DOUT], F32, name="brow")
            bbc = bp.tile([P, DOUT], F32, name="bbc")
            nc.gpsimd.dma_start(out=brow[:, :], in_=bi[:, :])
            nc.gpsimd.partition_broadcast(bbc[:, :], brow[:, :])

            # Per-k tiles give exact DMA->matmul deps: matmul (j,m,k) only
            # waits on x[k] and w[j][k].  dma_start dispatch costs ~640 ns
            # serially on the issuing sequencer, so the head is issue-bound:
            # keep tile loads monolithic (one dma_start each), interleaved
            # x/w in k order so the first matmul group's inputs dispatch
            # first.  (Splitting loads across more dma_starts was measured
            # strictly worse: the sequencer stalls on per-lane completion
            # waits once more than 8 DMAs are outstanding.)
            xtl = []
            w_first = []
            for k in range(KT):
                xk = xp.tile([P, TE], mm_dtype, name=f"xsb{k}")
                nc.sync.dma_start(out=xk[:, :], in_=xt[k * P : (k + 1) * P, :])
                xtl.append(xk)
                wkt = wp.tile([P, NFREE], mm_dtype, name="wsb")
                nc.sync.dma_start(out=wkt[:, :], in_=wt[k * P : (k + 1) * P, 0:NFREE])
                w_first.append(wkt)

            def x_slice(k, m):
                return xtl[k][:, m * P : (m + 1) * P]

            all_w = {0: w_first}

            def get_w(j):
                if j not in all_w:
                    nsl = slice(j * NFREE, (j + 1) * NFREE)
                    wtl = []
                    for k in range(KT):
                        wkt = wp.tile([P, NFREE], mm_dtype, name="wsb")
                        nc.sync.dma_start(
                            out=wkt[:, :], in_=wt[k * P : (k + 1) * P, nsl]
                        )
                        wtl.append(wkt)
                    all_w[j] = wtl
                return all_w[j]

            def _evict(j, m, ps, final=False):
                nsl = slice(j * NFREE, (j + 1) * NFREE)
                ysb = op.tile([P, NFREE], F32, name="ysb")
                # Stores go out on the ACT engine's DGE: a store waiting
                # on its eviction must not head-of-line-block the weight
                # loads issued on the Sync engine.  The last chunk's store
                # dispatches (~640 ns each, serial) are on the critical
                # tail — by then Sync has no loads left, so alternate the
                # final chunk's stores across both HWDGE engines, and
                # pipeline the very last eviction in halves.
                last_chunk = j == NCH - 1
                store_eng = (nc.sync if m % 2 else nc.scalar) if last_chunk else nc.scalar
                halves = 2 if final else 1
                step = NFREE // halves
                for h in range(halves):
                    fsl = slice(h * step, (h + 1) * step)
                    osl = slice(j * NFREE + h * step, j * NFREE + (h + 1) * step)
                    nc.vector.tensor_add(ysb[:, fsl], ps[:, fsl], bbc[:, osl])
                    store_eng.dma_start(
                        out=y[m * P : (m + 1) * P, osl], in_=ysb[:, fsl]
                    )

            for j in range(NCH):
                wtl = get_w(j)
                for m in range(MT):
                    ps = pp.tile([P, NFREE], F32, name="ps")
                    for k in range(KT):
                        nc.tensor.matmul(
                            ps[:, :],
                            lhsT=x_slice(k, m),
                            rhs=wtl[k][:, :],
                            start=(k == 0),
                            stop=(k == KT - 1),
                        )
                    _evict(j, m, ps, final=(j == NCH - 1 and m == MT - 1))

    nc.compile()
    return nc


def _get_nc(mm_dtype):
    key = str(mm_dtype)
    if key not in _NC_CACHE:
        _NC_CACHE[key] = _build_nc(mm_dtype)
    return _NC_CACHE[key]


def _run(x, weight, bias, expert_frequency, mm_dtype=MM_DTYPE, **run_kwargs):
    x = np.ascontiguousarray(np.asarray(x, dtype=np.float32))
    weight = np.ascontiguousarray(np.asarray(weight, dtype=np.float32))
    bias = np.ascontiguousarray(np.asarray(bias, dtype=np.float32))
    freq = np.asarray(expert_frequency, dtype=np.int64)

    assert x.shape == (T, DIN), x.shape
    assert weight.shape == (E, DOUT, DIN), weight.shape
    assert bias.shape == (E, DOUT), bias.shape
    assert freq.shape == (E,) and int(freq.sum()) == T
    # Tokens are pre-sorted by expert with balanced counts (the reference's
    # batched-matmul reshape requires this too).
    assert np.all(freq == TE), freq

    np_dt = _np_dtype(mm_dtype)
    xg = x.reshape(E, TE, DIN)
    in_maps = [
        {
            "xt": np.ascontiguousarray(xg[e].T).astype(np_dt),
            "wt": np.ascontiguousarray(weight[e].T).astype(np_dt),
            "bi": np.ascontiguousarray(bias[e][None, :]),
        }
        for e in range(E)
    ]
    nc = _get_nc(mm_dtype)
    out = run_bass_kernel_spmd(nc, in_maps, core_ids=list(range(E)), **run_kwargs)
    y = np.concatenate([r["y"] for r in out.results], axis=0)
    return y, out


def kernel(x, weight, bias, expert_frequency):
    y, _ = _run(x, weight, bias, expert_frequency)
    return y


# revision 37
# speedup vs baseline: 1.0029x; 1.0029x over previous
"""MoE grouped-GEMM (Experts) kernel for 8 Trainium2 NeuronCores.

Problem: x [8192, 1024] f32, tokens grouped contiguously by expert (1024
tokens per expert), weight [8, 4096, 1024], bias [8, 4096].  Output
y[t] = x[t] @ W[e(t)].T + b[e(t)], shape [8192, 4096].

Sharding: expert parallelism — expert e lives on core e.  Token dispatch is
a host-side slice (tokens are pre-sorted and balanced), so no collectives
are needed.  Each core runs one [1024x1024] @ [1024x4096] GEMM.

Device layout: the host feeds each core xT = x_e.T [DIN, TE] and
wT = W_e.T [DIN, DOUT] (both contiguous) so every DMA is wide-row
contiguous and the contraction dim K=DIN lands on SBUF partitions with no
on-device transposes.  Per core:
  - xT is SBUF-resident, wT streams through SBUF in 512-col chunks.
  - For each n-chunk j (512 cols) and each m-tile (128 tokens):
    8 accumulating matmuls over the K tiles into one PSUM bank,
    then a DVE add of the (partition-broadcast) bias evicts PSUM->SBUF,
    and the result DMAs out.  PSUM banks rotate (bufs=8) so the PE never
    stalls on eviction.

Matmul dtype: plain fp32 matmul on TRN2 is decomposed by the compiler
into 2 half-rate passes (4 cycles/row) — 4x slower than bf16.  float32r
(TF32-like rounding, fp32 storage) streams at 1 cycle/row for moving
free dim >= 256.  The matmul input dtype is configurable below.
"""

import sys

for _p in ("/opt/trn_rl_repo",):
    if _p not in sys.path:
        sys.path.insert(0, _p)

import numpy as np

import concourse.bass as bass  # noqa: F401  (registers lowerings)
import concourse.mybir as mybir
import concourse.tile as tile
from concourse import bacc
from concourse.bass_utils import run_bass_kernel_spmd

E = 8
T = 8192
DIN = 1024
DOUT = 4096
TE = T // E          # 1024 tokens per expert
P = 128
KT = DIN // P        # 8 k-tiles
MT = TE // P         # 8 m-tiles
NFREE = 512          # matmul moving free dim (one PSUM bank of fp32)
NCH = DOUT // NFREE  # 8 n-chunks
F32 = mybir.dt.float32

# Matmul input dtype for the device kernel.
MM_DTYPE = mybir.dt.float32r

_NC_CACHE = {}


def _np_dtype(mm_dtype):
    return mybir.dt.np(mm_dtype)


def _build_nc(mm_dtype):
    nc = bacc.Bacc(None, target_bir_lowering=False)

    xt = nc.dram_tensor("xt", [DIN, TE], mm_dtype, kind="ExternalInput")
    wt = nc.dram_tensor("wt", [DIN, DOUT], mm_dtype, kind="ExternalInput")
    bi = nc.dram_tensor("bi", [1, DOUT], F32, kind="ExternalInput")
    y = nc.dram_tensor("y", [TE, DOUT], F32, kind="ExternalOutput")

    with tile.TileContext(nc) as tc:
        with (
            tc.tile_pool(name="xp", bufs=1) as xp,
            tc.tile_pool(name="wp", bufs=3 * KT) as wp,
            tc.tile_pool(name="bp", bufs=1) as bp,
            tc.tile_pool(name="op", bufs=4) as op,
            tc.tile_pool(name="pp", bufs=8, space="PSUM") as pp,
        ):
            # Bias: one row DMA + partition broadcast to all 128 partitions.
            # Loaded via the GPSIMD SWDGE so it stays off the Sync engine's
            # serial dma_start dispatch path (~640 ns each); the broadcast
            # result is only needed at the first PSUM eviction (~16 us in).
            brow = bp.tile([1, DOUT], F32, name="brow")
            bbc = bp.tile([P, DOUT], F32, name="bbc")
            nc.gpsimd.dma_start(out=brow[:, :], in_=bi[:, :])
            nc.gpsimd.partition_broadcast(bbc[:, :], brow[:, :])

            # Per-k tiles give exact DMA->matmul deps: matmul (j,m,k) only
            # waits on x[k] and w[j][k].  dma_start dispatch costs ~640 ns
            # serially on the issuing sequencer, so the head is issue-bound:
            # keep tile loads monolithic (one dma_start each), interleaved
            # x/w in k order so the first matmul group's inputs dispatch
            # first.  (Splitting loads across more dma_starts was measured
            # strictly worse: the sequencer stalls on per-lane completion
            # waits once more than 8 DMAs are outstanding.)
            xtl = []
            w_first = []
            for k in range(KT):
                xk = xp.tile([P, TE], mm_dtype, name=f"xsb{k}")
                nc.sync.dma_start(out=xk[:, :], in_=xt[k * P : (k + 1) * P, :])
                xtl.append(xk)
                wkt = wp.tile([P, NFREE], mm_dtype, name="wsb")
                nc.sync.dma_start(out=wkt[:, :], in_=wt[k * P : (k + 1) * P, 0:NFREE])
                w_first.append(wkt)

            def x_slice(k, m):
                return xtl[k][:, m * P : (m + 1) * P]

            all_w = {0: w_first}

            def get_w(j):
                if j not in all_w:
                    nsl = slice(j * NFREE, (j + 1) * NFREE)
                    wtl = []
                    for k in range(KT):
                        wkt = wp.tile([P, NFREE], mm_dtype, name="wsb")
                        nc.sync.dma_start(
                            out=wkt[:, :], in_=wt[k * P : (k + 1) * P, nsl]
                        )
                        wtl.append(wkt)
                    all_w[j] = wtl
                return all_w[j]

            def _evict(j, m, ps, final=False):
                nsl = slice(j * NFREE, (j + 1) * NFREE)
                ysb = op.tile([P, NFREE], F32, name="ysb")
                # Stores go out on the ACT engine's DGE: a store waiting
                # on its eviction must not head-of-line-block the weight
                # loads issued on the Sync engine.  The last chunk's store
                # dispatches (~640 ns each, serial) are on the critical
                # tail — by then Sync has no loads left, so alternate the
                # final chunk's stores across both HWDGE engines, and
                # pipeline the very last eviction in halves.
                last_chunk = j == NCH - 1
                store_eng = (nc.sync if m % 2 else nc.scalar) if last_chunk else nc.scalar
                halves = 2 if final else 1
                step = NFREE // halves
                for h in range(halves):
                    fsl = slice(h * step, (h + 1) * step)
                    osl = slice(j * NFREE + h * step, j * NFREE + (h + 1) * step)
                    nc.vector.tensor_add(ysb[:, fsl], ps[:, fsl], bbc[:, osl])
                    store_eng.dma_start(
                        out=y[m * P : (m + 1) * P, osl], in_=ysb[:, fsl]
                    )

            for j in range(NCH):
                wtl = get_w(j)
                for m in range(MT):
                    ps = pp.tile([P, NFREE], F32, name="ps")
                    for k in range(KT):
                        nc.tensor.matmul(
                            ps[:, :],
                            lhsT=x_slice(k, m),
                            rhs=wtl[k][:, :],
                            start=(k == 0),
                            stop=(k == KT - 1),
                        )
                    _evict(j, m, ps, final=(j == NCH - 1 and m == MT - 1))

    nc.compile()
    return nc


def _get_nc(mm_dtype):
    key = str(mm_dtype)
    if key not in _NC_CACHE:
        _NC_CACHE[key] = _build_nc(mm_dtype)
    return _NC_CACHE[key]


def _run(x, weight, bias, expert_frequency, mm_dtype=MM_DTYPE, **run_kwargs):
    x = np.ascontiguousarray(np.asarray(x, dtype=np.float32))
    weight = np.ascontiguousarray(np.asarray(weight, dtype=np.float32))
    bias = np.ascontiguousarray(np.asarray(bias, dtype=np.float32))
    freq = np.asarray(expert_frequency, dtype=np.int64)

    assert x.shape == (T, DIN), x.shape
    assert weight.shape == (E, DOUT, DIN), weight.shape
    assert bias.shape == (E, DOUT), bias.shape
    assert freq.shape == (E,) and int(freq.sum()) == T
    # Tokens are pre-sorted by expert with balanced counts (the reference's
    # batched-matmul reshape requires this too).
    assert np.all(freq == TE), freq

    np_dt = _np_dtype(mm_dtype)
    xg = x.reshape(E, TE, DIN)
    in_maps = [
        {
            "xt": np.ascontiguousarray(xg[e].T).astype(np_dt),
            "wt": np.ascontiguousarray(weight[e].T).astype(np_dt),
            "bi": np.ascontiguousarray(bias[e][None, :]),
        }
        for e in range(E)
    ]
    nc = _get_nc(mm_dtype)
    out = run_bass_kernel_spmd(nc, in_maps, core_ids=list(range(E)), **run_kwargs)
    y = np.concatenate([r["y"] for r in out.results], axis=0)
    return y, out


def kernel(x, weight, bias, expert_frequency):
    y, _ = _run(x, weight, bias, expert_frequency)
    return y


# revision 38
# speedup vs baseline: 1.0838x; 1.0807x over previous
"""MoE grouped-GEMM (Experts) kernel for 8 Trainium2 NeuronCores.

Problem: x [8192, 1024] f32, tokens grouped contiguously by expert (1024
tokens per expert), weight [8, 4096, 1024], bias [8, 4096].  Output
y[t] = x[t] @ W[e(t)].T + b[e(t)], shape [8192, 4096].

Sharding: expert parallelism — expert e lives on core e.  Token dispatch is
a host-side slice (tokens are pre-sorted and balanced), so no collectives
are needed.  Each core runs one [1024x1024] @ [1024x4096] GEMM.

Device layout: the host feeds each core xT = x_e.T [DIN, TE] and
wT = W_e.T [DIN, DOUT] (both contiguous) so every DMA is wide-row
contiguous and the contraction dim K=DIN lands on SBUF partitions with no
on-device transposes.  Per core:
  - xT is SBUF-resident, wT streams through SBUF in 512-col chunks.
  - For each n-chunk j (512 cols) and each m-tile (128 tokens):
    8 accumulating matmuls over the K tiles into one PSUM bank,
    then a DVE add of the (partition-broadcast) bias evicts PSUM->SBUF,
    and the result DMAs out.  PSUM banks rotate (bufs=8) so the PE never
    stalls on eviction.

Matmul dtype: plain fp32 matmul on TRN2 is decomposed by the compiler
into 2 half-rate passes (4 cycles/row) — 4x slower than bf16.  float32r
(TF32-like rounding, fp32 storage) streams at 1 cycle/row for moving
free dim >= 256.  The matmul input dtype is configurable below.
"""

import sys

for _p in ("/opt/trn_rl_repo",):
    if _p not in sys.path:
        sys.path.insert(0, _p)

import numpy as np

import concourse.bass as bass  # noqa: F401  (registers lowerings)
import concourse.mybir as mybir
import concourse.tile as tile
from concourse import bacc
from concourse.bass_utils import run_bass_kernel_spmd

E = 8
T = 8192
DIN = 1024
DOUT = 4096
TE = T // E          # 1024 tokens per expert
P = 128
KT = DIN // P        # 8 k-tiles
MT = TE // P         # 8 m-tiles
NFREE = 512          # matmul moving free dim (one PSUM bank of fp32)
NCH = DOUT // NFREE  # 8 n-chunks
F32 = mybir.dt.float32

# Matmul input dtype for the device kernel.  fp16 streams at 1 cycle/row
# with FWL-accelerated LDWEIGHTS (the fp32-family weight path cannot use
# FWL), measuring ~130 us vs float32r's ~140 us on this structure, at
# 2.5e-4 relative error (fp32 accumulation in PSUM; inputs are in range
# for fp16: |x| < ~6, |w| ~ 0.02-scale).  Set to mybir.dt.float32r for
# 1.25e-4 error at ~140 us, or mybir.dt.float32 for bit-exact at ~470 us.
MM_DTYPE = mybir.dt.float16

_NC_CACHE = {}


def _np_dtype(mm_dtype):
    return mybir.dt.np(mm_dtype)


def _build_nc(mm_dtype):
    nc = bacc.Bacc(None, target_bir_lowering=False)

    xt = nc.dram_tensor("xt", [DIN, TE], mm_dtype, kind="ExternalInput")
    wt = nc.dram_tensor("wt", [DIN, DOUT], mm_dtype, kind="ExternalInput")
    bi = nc.dram_tensor("bi", [1, DOUT], F32, kind="ExternalInput")
    y = nc.dram_tensor("y", [TE, DOUT], F32, kind="ExternalOutput")

    with tile.TileContext(nc) as tc:
        with (
            tc.tile_pool(name="xp", bufs=1) as xp,
            tc.tile_pool(name="wp", bufs=3 * KT) as wp,
            tc.tile_pool(name="bp", bufs=1) as bp,
            tc.tile_pool(name="op", bufs=4) as op,
            tc.tile_pool(name="pp", bufs=8, space="PSUM") as pp,
        ):
            # Bias: one row DMA + partition broadcast to all 128 partitions.
            # Loaded via the GPSIMD SWDGE so it stays off the Sync engine's
            # serial dma_start dispatch path (~640 ns each); the broadcast
            # result is only needed at the first PSUM eviction (~16 us in).
            brow = bp.tile([1, DOUT], F32, name="brow")
            bbc = bp.tile([P, DOUT], F32, name="bbc")
            nc.gpsimd.dma_start(out=brow[:, :], in_=bi[:, :])
            nc.gpsimd.partition_broadcast(bbc[:, :], brow[:, :])

            # Per-k tiles give exact DMA->matmul deps: matmul (j,m,k) only
            # waits on x[k] and w[j][k].  dma_start dispatch costs ~640 ns
            # serially on the issuing sequencer, so the head is issue-bound:
            # keep tile loads monolithic (one dma_start each), interleaved
            # x/w in k order so the first matmul group's inputs dispatch
            # first.  (Splitting loads across more dma_starts was measured
            # strictly worse: the sequencer stalls on per-lane completion
            # waits once more than 8 DMAs are outstanding.)
            xtl = []
            w_first = []
            for k in range(KT):
                xk = xp.tile([P, TE], mm_dtype, name=f"xsb{k}")
                nc.sync.dma_start(out=xk[:, :], in_=xt[k * P : (k + 1) * P, :])
                xtl.append(xk)
                wkt = wp.tile([P, NFREE], mm_dtype, name="wsb")
                nc.sync.dma_start(out=wkt[:, :], in_=wt[k * P : (k + 1) * P, 0:NFREE])
                w_first.append(wkt)

            def x_slice(k, m):
                return xtl[k][:, m * P : (m + 1) * P]

            all_w = {0: w_first}

            def get_w(j):
                if j not in all_w:
                    nsl = slice(j * NFREE, (j + 1) * NFREE)
                    wtl = []
                    for k in range(KT):
                        wkt = wp.tile([P, NFREE], mm_dtype, name="wsb")
                        nc.sync.dma_start(
                            out=wkt[:, :], in_=wt[k * P : (k + 1) * P, nsl]
                        )
                        wtl.append(wkt)
                    all_w[j] = wtl
                return all_w[j]

            def _evict(j, m, ps, final=False):
                nsl = slice(j * NFREE, (j + 1) * NFREE)
                ysb = op.tile([P, NFREE], F32, name="ysb")
                # Stores go out on the ACT engine's DGE: a store waiting
                # on its eviction must not head-of-line-block the weight
                # loads issued on the Sync engine.  The last chunk's store
                # dispatches (~640 ns each, serial) are on the critical
                # tail — by then Sync has no loads left, so alternate the
                # final chunk's stores across both HWDGE engines, and
                # pipeline the very last eviction in halves.
                last_chunk = j == NCH - 1
                store_eng = (nc.sync if m % 2 else nc.scalar) if last_chunk else nc.scalar
                halves = 2 if final else 1
                step = NFREE // halves
                for h in range(halves):
                    fsl = slice(h * step, (h + 1) * step)
                    osl = slice(j * NFREE + h * step, j * NFREE + (h + 1) * step)
                    nc.vector.tensor_add(ysb[:, fsl], ps[:, fsl], bbc[:, osl])
                    store_eng.dma_start(
                        out=y[m * P : (m + 1) * P, osl], in_=ysb[:, fsl]
                    )

            for j in range(NCH):
                wtl = get_w(j)
                for m in range(MT):
                    ps = pp.tile([P, NFREE], F32, name="ps")
                    for k in range(KT):
                        nc.tensor.matmul(
                            ps[:, :],
                            lhsT=x_slice(k, m),
                            rhs=wtl[k][:, :],
                            start=(k == 0),
                            stop=(k == KT - 1),
                        )
                    _evict(j, m, ps, final=(j == NCH - 1 and m == MT - 1))

    nc.compile()
    return nc


def _get_nc(mm_dtype):
    key = str(mm_dtype)
    if key not in _NC_CACHE:
        _NC_CACHE[key] = _build_nc(mm_dtype)
    return _NC_CACHE[key]


def _run(x, weight, bias, expert_frequency, mm_dtype=MM_DTYPE, **run_kwargs):
    x = np.ascontiguousarray(np.asarray(x, dtype=np.float32))
    weight = np.ascontiguousarray(np.asarray(weight, dtype=np.float32))
    bias = np.ascontiguousarray(np.asarray(bias, dtype=np.float32))
    freq = np.asarray(expert_frequency, dtype=np.int64)

    assert x.shape == (T, DIN), x.shape
    assert weight.shape == (E, DOUT, DIN), weight.shape
    assert bias.shape == (E, DOUT), bias.shape
    assert freq.shape == (E,) and int(freq.sum()) == T
    # Tokens are pre-sorted by expert with balanced counts (the reference's
    # batched-matmul reshape requires this too).
    assert np.all(freq == TE), freq

    np_dt = _np_dtype(mm_dtype)
    xg = x.reshape(E, TE, DIN)
    in_maps = [
        {
            "xt": np.ascontiguousarray(xg[e].T).astype(np_dt),
            "wt": np.ascontiguousarray(weight[e].T).astype(np_dt),
            "bi": np.ascontiguousarray(bias[e][None, :]),
        }
        for e in range(E)
    ]
    nc = _get_nc(mm_dtype)
    out = run_bass_kernel_spmd(nc, in_maps, core_ids=list(range(E)), **run_kwargs)
    y = np.concatenate([r["y"] for r in out.results], axis=0)
    return y, out


def kernel(x, weight, bias, expert_frequency):
    y, _ = _run(x, weight, bias, expert_frequency)
    return y
